# revision 25
# baseline (speedup 1.0000x reference)
"""Distributed Trainium2 kernel for nn_Attention_14181982012033.

Math (reference): p = x @ W; per-head ph = split(p); q = ph/sqrt(d);
logits = q @ ph^T; w = softmax(logits); attn = w @ ph; out = merge(attn) @ W.
Shapes: x [4, 2048, 1024] f32, W [1024, 1024] f32, 16 heads, d = 64.

Sharding (zero collectives): 8 cores = 4 batches x 2 query-halves. Each core
receives xT = x[b]^T (bf16, host-pre-transposed) and W (bf16), computes the
full projection pT = (xW)^T for its batch, derives the natural-layout values
from pT via PE identity-matmul transposes (instead of a second full
projection), runs attention for its 1024 query rows over all 2048 keys,
applies the output projection, and writes a [1024, 1024] f32 slab.

SPMD trick: both cores of a batch run the IDENTICAL graph. Core 2b+1's xT is
rolled by -1024 along S, so "query rows" are always pT[:, 0:1024]; softmax
over keys is permutation-invariant, so rolled keys give identical output.

The attention inner loop is ACT(exp)-bound (~2.3 us/kt: 2 exps of
[128,1024] at ~1.15 us vs ~1.8 us of PE matmul+fill work), so all PE-side
projection/transpose work is interleaved into the loop's slack.

Softmax denominator trick: p_pad tiles are laid out [O vA vB][O vA vB]...[O]
with O = 64 ones columns shared across adjacent pairs. Head A's AV lhsT is
[O|vA] (denominator lands replicated in PSUM rows 0:64, numerators in
64:128); head B's is [vB|O'] (numerators 0:64, denominator 64:128). The
epilogue is then just two DVE ops per pair (no DRAM roundtrip, no
broadcast): attnT = numer * reciprocal(denom-replicated).
"""

import os
import sys
from contextlib import ExitStack

import numpy as np

for _p in ("/opt/trn_rl_repo", "/opt/pypackages"):
    if _p not in sys.path:
        sys.path.append(_p)

import ml_dtypes

import concourse.bass as bass
import concourse.bacc as bacc
import concourse.mybir as mybir
import concourse.tile as tile
from concourse.bass_utils import run_bass_kernel_spmd

B, S, H, NH, D = 4, 2048, 1024, 16, 64
Q = 1024          # query rows per core
KT = H // 128     # 8 partition tiles along H
ST = S // 128     # 16 partition tiles along S
NP = NH // 2      # 8 head-pairs
PW = 192          # pair stride in p_pad: [ones(64) | vA(64) | vB(64)]
PPW = NP * PW + 64  # 1600 cols per p_pad tile (trailing ones block)
DT = mybir.dt.bfloat16
F32 = mybir.dt.float32
SCALE = 1.0 / float(np.sqrt(D))
AV_SKEW = 4

_CACHE = {}


def _build():
    nc = bacc.Bacc()
    xT_d = nc.declare_dram_parameter("xT", [H, S], DT, isOutput=False)
    W_d = nc.declare_dram_parameter("W", [H, H], DT, isOutput=False)
    I_d = nc.declare_dram_parameter("I", [128, 128], DT, isOutput=False)
    out_d = nc.declare_dram_parameter("out", [Q, H], F32, isOutput=True)

    with ExitStack() as ctx:
        tc = ctx.enter_context(tile.TileContext(nc))
        res = ctx.enter_context(tc.tile_pool(name="res", bufs=1))
        work = ctx.enter_context(tc.tile_pool(name="work", bufs=3))
        evac = ctx.enter_context(tc.tile_pool(name="evac", bufs=4))
        psg = ctx.enter_context(tc.tile_pool(name="psg", bufs=2, space="PSUM"))
        psav = ctx.enter_context(tc.tile_pool(name="psav", bufs=1, space="PSUM"))

        # ---- load inputs. xT on the sync DGE ring, W on the act ring so the
        # two streams run in parallel; per-k interleave lets the first pT
        # projection group accumulate k-chunks as they land.
        ident = res.tile([128, 128], DT, tag="I", name="I")
        nc.sync.dma_start(out=ident[:], in_=I_d[:, :])
        # preload the exp table set while input DMAs stream (first real exp
        # would otherwise pay the ~2.7us ACT_TABLE_LOAD on the critical path)
        warm = work.tile([1, 128], F32, tag="warm", name="warm", bufs=1)
        nc.scalar.activation(out=warm[:], in_=ident[0:1, :],
                             func=mybir.ActivationFunctionType.Exp, scale=1.0)
        # Inputs land in dependency order across both hwdge rings: the
        # columns the first projection groups touch come first.
        xT = [res.tile([128, S], DT, tag=f"xT{i}", name=f"xT{i}")
              for i in range(KT)]
        Wt = [res.tile([128, H], DT, tag=f"W{i}", name=f"W{i}")
              for i in range(KT)]
        for i in range(KT):
            qx = nc.sync if i % 2 == 0 else nc.scalar
            qw = nc.scalar if i % 2 == 0 else nc.sync
            qx.dma_start(out=xT[i][:, 0:1024], in_=xT_d[i * 128:(i + 1) * 128, 0:1024])
            qw.dma_start(out=Wt[i][:, 0:256], in_=W_d[i * 128:(i + 1) * 128, 0:256])
        for i in range(KT):
            qx = nc.sync if i % 2 == 0 else nc.scalar
            qw = nc.scalar if i % 2 == 0 else nc.sync
            qx.dma_start(out=xT[i][:, 1024:2048],
                         in_=xT_d[i * 128:(i + 1) * 128, 1024:2048])
            qw.dma_start(out=Wt[i][:, 256:1024],
                         in_=W_d[i * 128:(i + 1) * 128, 256:1024])

        # ---- pT = (x @ W)^T : [H, S] bf16, 8 tiles of [128, S]
        pT = [res.tile([128, S], DT, tag=f"pT{i}", name=f"pT{i}") for i in range(KT)]

        def proj_pT_group(f, sc):
            ps = psg.tile([128, 1024], F32, tag="g", name="pjg")
            for k in range(KT):
                nc.tensor.matmul(
                    out=ps[:, 0:512],
                    lhsT=Wt[k][:, f * 128:(f + 1) * 128],
                    rhs=xT[k][:, sc * 512:(sc + 1) * 512],
                    start=(k == 0),
                    stop=(k == KT - 1),
                )
            nc.vector.tensor_copy(
                out=pT[f][:, sc * 512:(sc + 1) * 512], in_=ps[:, 0:512]
            )

        # ---- p natural (values), derived from pT by PE transpose.
        # One big tile [128, ST*PPW]; tile st occupies cols st*PPW:(st+1)*PPW
        # with layout [O vA0 vB0 O vA1 vB1 ... O] (O = 64 ones cols).
        p_pad = res.tile([128, ST * PPW], DT, tag="pp", name="pp")
        for st in range(ST):
            main = p_pad[:, st * PPW:st * PPW + NP * PW].rearrange(
                "p (g u) -> p g u", u=PW
            )[:, :, 0:64]
            nc.vector.memset(main, 1.0)
            nc.vector.memset(p_pad[:, st * PPW + NP * PW:(st + 1) * PPW], 1.0)

        def transpose_pack(fp, half):
            # transpose 8 pT[fp] key-chunks (st = 8*half .. 8*half+7) into
            # their p_pad [vA|vB] slots via the DMA XBAR transpose — fully
            # off the PE/DVE engines (DMA queues are nearly idle mid-run).
            for j in range(8):
                st = 8 * half + j
                dst = p_pad[:, st * PPW + fp * PW + 64:st * PPW + fp * PW + 192]
                nc.sync.dma_start(out=dst,
                                  in_=pT[fp][:, st * 128:(st + 1) * 128],
                                  transpose=True)

        # pT[0] q-side chunks precede the loop; k-side chunks and all
        # transposes stream just-in-time inside the attention loop. The
        # first group's k-chain paces on input DMAs, so discarded filler
        # matmuls (on already-landed data) are woven in to keep the PE busy
        # enough that the HAM clock gate releases before attention starts.
        fill_ps = psg.tile([128, 1024], F32, tag="g", name="fill")
        ps00 = psg.tile([128, 1024], F32, tag="g", name="pjg")
        for k in range(KT):
            nc.tensor.matmul(
                out=ps00[:, 0:512],
                lhsT=Wt[k][:, 0:128],
                rhs=xT[k][:, 0:512],
                start=(k == 0),
                stop=(k == KT - 1),
            )
            if k < KT - 1:
                for h in range(2):
                    nc.tensor.matmul(out=fill_ps[:, 0:512], lhsT=ident[:],
                                     rhs=xT[0][:, h * 512:(h + 1) * 512],
                                     start=True, stop=True)
        nc.vector.tensor_copy(out=pT[0][:, 0:512], in_=ps00[:, 0:512])
        proj_pT_group(0, 1)

        # ---- attention: one continuous software-pipelined stream over all
        # (pair, kt) steps. Crossing a pair boundary, the old pair's last AV
        # accumulations and epilogue interleave into the new pair's first
        # gram/exp iterations, so the ACT engine (the bottleneck) never
        # drains. AV lags gram/exp by AV_SKEW steps.
        attnT = [res.tile([128, Q], DT, tag=f"at{i}", name=f"at{i}") for i in range(KT)]

        def do_av(eA, eB, fp, kt, av0, av1):
            st0, sp0 = (kt == 0), (kt == ST - 1)
            base = kt * PPW + fp * PW
            wA = p_pad[:, base:base + 128]            # [O | vA]
            wB = p_pad[:, base + 128:base + 256]      # [vB | O']
            nc.tensor.matmul(out=av0[:, 0:512], lhsT=wA,
                             rhs=eA[:, 0:512], start=st0, stop=sp0)
            nc.tensor.matmul(out=av1[:, 0:512], lhsT=wB,
                             rhs=eB[:, 0:512], start=st0, stop=sp0)
            nc.tensor.matmul(out=av0[:, 512:1024], lhsT=wA,
                             rhs=eA[:, 512:1024], start=st0, stop=sp0)
            nc.tensor.matmul(out=av1[:, 512:1024], lhsT=wB,
                             rhs=eB[:, 512:1024], start=st0, stop=sp0)

        def epilogue(fp, av0, av1):
            # denominators are replicated across 64 PSUM rows (the
            # ones-block columns of the AV lhsT): normalize is reciprocal +
            # multiply straight out of PSUM. reciprocal_approx_fast
            # mis-reads inputs at a nonzero base partition (reads base 0),
            # so D_B is staged down to base 0 first. (AluOpType.divide
            # fails walrus codegen - do not use.)
            rcA = work.tile([64, 1024], F32, tag="rcA", name="rcA", bufs=1)
            rcB = work.tile([64, 1024], F32, tag="rcB", name="rcB", bufs=1)
            dB = work.tile([64, 1024], F32, tag="dB", name="dB", bufs=1)
            nc.vector.reciprocal_approx_fast(out=rcA[:], in_=av0[0:64, :])
            nc.vector.tensor_tensor(out=attnT[fp][0:64, :], in0=av0[64:128, :],
                                    in1=rcA[:], op=mybir.AluOpType.mult)
            nc.vector.tensor_copy(out=dB[:], in_=av1[64:128, :])
            nc.vector.reciprocal_approx_fast(out=rcB[:], in_=dB[:])
            nc.vector.tensor_tensor(out=attnT[fp][64:128, :], in0=av1[0:64, :],
                                    in1=rcB[:], op=mybir.AluOpType.mult)

        pending = []
        av_cur = None
        for g in range(NP * ST + AV_SKEW + 1):
            fp, kt = divmod(g, ST)
            # AV for step g - AV_SKEW first: its e-inputs are long ready, so
            # these MMs stream without waits, and keeping them ahead of the
            # grams in program order avoids extra gram<->AV PE transitions.
            if len(pending) > AV_SKEW or g >= NP * ST:
                if pending:
                    args = pending.pop(0)
                    do_av(*args)
                    if args[3] == ST - 1:          # pair args[2] finished
                        epilogue(args[2], args[4], args[5])
            if g < NP * ST:
                if kt == 0:
                    # pair fp's AV accumulators. The previous pair's
                    # epilogue (the last reader of the old tiles) was
                    # emitted at step fp*ST + AV_SKEW - 1 above, before the
                    # first do_av of this pair needs the fresh tiles.
                    av_cur = (
                        psav.tile([128, 1024], F32, tag="av0", name="av0"),
                        psav.tile([128, 1024], F32, tag="av1", name="av1"),
                    )
                ks = slice(kt * 128, (kt + 1) * 128)
                tA = psg.tile([128, 1024], F32, tag="g", name="gA")
                tB = psg.tile([128, 1024], F32, tag="g", name="gB")
                nc.tensor.matmul(out=tA[:, 0:512], lhsT=pT[fp][0:64, ks],
                                 rhs=pT[fp][0:64, 0:512], start=True, stop=True)
                nc.tensor.matmul(out=tA[:, 512:1024], lhsT=pT[fp][0:64, ks],
                                 rhs=pT[fp][0:64, 512:1024], start=True, stop=True)
                eA = work.tile([128, 1024], DT, tag="eA", name="eA",
                               bufs=AV_SKEW + 2)
                nc.scalar.activation(out=eA[:], in_=tA[:],
                                     func=mybir.ActivationFunctionType.Exp,
                                     scale=SCALE)
                nc.tensor.matmul(out=tB[:, 0:512], lhsT=pT[fp][64:128, ks],
                                 rhs=pT[fp][64:128, 0:512], start=True, stop=True)
                nc.tensor.matmul(out=tB[:, 512:1024], lhsT=pT[fp][64:128, ks],
                                 rhs=pT[fp][64:128, 512:1024], start=True, stop=True)
                eB = work.tile([128, 1024], DT, tag="eB", name="eB",
                               bufs=AV_SKEW + 2)
                nc.scalar.activation(out=eB[:], in_=tB[:],
                                     func=mybir.ActivationFunctionType.Exp,
                                     scale=SCALE)
                pending.append((eA, eB, fp, kt, av_cur[0], av_cur[1]))
            # ---- fill PE slack with projection / transpose work
            if g >= NP * ST:
                continue
            if fp == 0:
                # pair 0's first AV_SKEW steps have no AV work yet - extra
                # PE slack hosts pair 1's projection early.
                if kt == 0:
                    proj_pT_group(0, 2)
                elif kt == 1:
                    transpose_pack(0, 0)
                elif kt == 2:
                    proj_pT_group(0, 3)
                elif kt == 3:
                    proj_pT_group(1, 0)
                elif kt == 5:
                    transpose_pack(0, 1)
                elif kt in (6, 8, 10):
                    proj_pT_group(1, (kt - 4) // 2)
                elif kt == 12:
                    transpose_pack(1, 0)
                elif kt == 13:
                    proj_pT_group(2, 0)
                elif kt == 14:
                    transpose_pack(1, 1)
                elif kt == 15:
                    proj_pT_group(2, 1)
            elif fp == 1:
                if kt in (2, 5):
                    proj_pT_group(2, (kt + 4) // 3)
                elif kt == 12:
                    transpose_pack(2, 0)
                elif kt == 14:
                    transpose_pack(2, 1)
            elif fp + 1 < NP:
                if kt in (1, 4, 7, 10):
                    proj_pT_group(fp + 1, (kt - 1) // 3)
                elif kt == 12:
                    transpose_pack(fp + 1, 0)
                elif kt == 14:
                    transpose_pack(fp + 1, 1)

        # ---- output projection: out[q, :] = attnc @ W
        for qt in range(Q // 128):
            for fc in range(H // 512):
                ps = psg.tile([128, 1024], F32, tag="g", name="opg")
                for k in range(KT):
                    nc.tensor.matmul(
                        out=ps[:, 0:512],
                        lhsT=attnT[k][:, qt * 128:(qt + 1) * 128],
                        rhs=Wt[k][:, fc * 512:(fc + 1) * 512],
                        start=(k == 0),
                        stop=(k == KT - 1),
                    )
                ot = evac.tile([128, 512], F32, tag="ot")
                nc.vector.tensor_copy(out=ot[:], in_=ps[:, 0:512])
                nc.sync.dma_start(
                    out=out_d[qt * 128:(qt + 1) * 128, fc * 512:(fc + 1) * 512],
                    in_=ot[:],
                )
    nc.finalize()
    return nc


def _get_nc():
    if "nc" not in _CACHE:
        _CACHE["nc"] = _build()
    return _CACHE["nc"]


def _install_ntff_hook():
    """Register the axon NTFF profiling hook if this image's antenv lacks
    ``axon_hooks`` (test/profiling path only; grading never hits this)."""
    import types

    try:
        from antenv.axon_hooks import get_axon_ntff_profile_hook  # noqa: F401
        return
    except ImportError:
        pass
    import antenv

    mod = types.ModuleType("antenv.axon_hooks")
    state = {"hook": None}
    mod.set_axon_ntff_profile_hook = lambda h: state.__setitem__("hook", h)
    mod.get_axon_ntff_profile_hook = lambda: state["hook"]
    sys.modules["antenv.axon_hooks"] = mod
    antenv.axon_hooks = mod
    try:
        from trn_agent_boot.trn_boot import _ntff_profile_via_ctypes

        hook = _ntff_profile_via_ctypes("/opt/axon/libaxon_pjrt.so")
        mod.set_axon_ntff_profile_hook(hook)
    except Exception as e:  # degrade: tracing skipped, run still works
        print(f"ntff hook install failed: {e}", file=sys.stderr)


def _run(x, W, trace=False):
    if trace:
        _install_ntff_hook()
    nc = _get_nc()
    bf = ml_dtypes.bfloat16
    Wb = np.ascontiguousarray(W.astype(bf))
    Ib = np.eye(128, dtype=bf)
    in_maps = []
    for c in range(8):
        b, half = divmod(c, 2)
        key = ("xT", b, half)
        if key not in _CACHE:
            xTb = np.ascontiguousarray(x[b].T).astype(bf)
            if half:
                xTb = np.ascontiguousarray(np.roll(xTb, -Q, axis=1))
            _CACHE[key] = xTb
        in_maps.append({"xT": _CACHE[key], "W": Wb, "I": Ib})
    try:
        r = run_bass_kernel_spmd(
            nc, in_maps, core_ids=list(range(8)), trace=trace
        )
    finally:
        for c in range(8):
            _CACHE.pop(("xT", c // 2, c % 2), None)
    y = np.empty((B, S, H), np.float32)
    for c in range(8):
        b, half = divmod(c, 2)
        y[b, half * Q:(half + 1) * Q, :] = r.results[c]["out"]
    _CACHE["last_result"] = r
    return y


def kernel(x, W):
    return _run(np.asarray(x, dtype=np.float32), np.asarray(W, dtype=np.float32),
                trace=bool(os.environ.get("BASS_KERNEL_TRACE")))


# revision 26
# speedup vs baseline: 1.0167x; 1.0167x over previous
"""Distributed Trainium2 kernel for nn_Attention_14181982012033.

Math (reference): p = x @ W; per-head ph = split(p); q = ph/sqrt(d);
logits = q @ ph^T; w = softmax(logits); attn = w @ ph; out = merge(attn) @ W.
Shapes: x [4, 2048, 1024] f32, W [1024, 1024] f32, 16 heads, d = 64.

Sharding (zero collectives): 8 cores = 4 batches x 2 query-halves. Each core
receives xT = x[b]^T (bf16, host-pre-transposed) and W (bf16), computes the
full projection pT = (xW)^T for its batch, derives the natural-layout values
from pT via PE identity-matmul transposes (instead of a second full
projection), runs attention for its 1024 query rows over all 2048 keys,
applies the output projection, and writes a [1024, 1024] f32 slab.

SPMD trick: both cores of a batch run the IDENTICAL graph. Core 2b+1's xT is
rolled by -1024 along S, so "query rows" are always pT[:, 0:1024]; softmax
over keys is permutation-invariant, so rolled keys give identical output.

The attention inner loop is ACT(exp)-bound (~2.3 us/kt: 2 exps of
[128,1024] at ~1.15 us vs ~1.8 us of PE matmul+fill work), so all PE-side
projection/transpose work is interleaved into the loop's slack.

Softmax denominator trick: p_pad tiles are laid out [O vA vB][O vA vB]...[O]
with O = 64 ones columns shared across adjacent pairs. Head A's AV lhsT is
[O|vA] (denominator lands replicated in PSUM rows 0:64, numerators in
64:128); head B's is [vB|O'] (numerators 0:64, denominator 64:128). The
epilogue is then just two DVE ops per pair (no DRAM roundtrip, no
broadcast): attnT = numer * reciprocal(denom-replicated).
"""

import os
import sys
from contextlib import ExitStack

import numpy as np

for _p in ("/opt/trn_rl_repo", "/opt/pypackages"):
    if _p not in sys.path:
        sys.path.append(_p)

import ml_dtypes

import concourse.bass as bass
import concourse.bacc as bacc
import concourse.mybir as mybir
import concourse.tile as tile
from concourse.bass_utils import run_bass_kernel_spmd

B, S, H, NH, D = 4, 2048, 1024, 16, 64
Q = 1024          # query rows per core
KT = H // 128     # 8 partition tiles along H
ST = S // 128     # 16 partition tiles along S
NP = NH // 2      # 8 head-pairs
PW = 192          # pair stride in p_pad: [ones(64) | vA(64) | vB(64)]
PPW = NP * PW + 64  # 1600 cols per p_pad tile (trailing ones block)
DT = mybir.dt.bfloat16
F32 = mybir.dt.float32
SCALE = 1.0 / float(np.sqrt(D))
AV_SKEW = 4

_CACHE = {}


def _build():
    nc = bacc.Bacc()
    xT_d = nc.declare_dram_parameter("xT", [H, S], DT, isOutput=False)
    W_d = nc.declare_dram_parameter("W", [H, H], DT, isOutput=False)
    I_d = nc.declare_dram_parameter("I", [128, 128], DT, isOutput=False)
    out_d = nc.declare_dram_parameter("out", [Q, H], F32, isOutput=True)

    with ExitStack() as ctx:
        tc = ctx.enter_context(tile.TileContext(nc))
        res = ctx.enter_context(tc.tile_pool(name="res", bufs=1))
        work = ctx.enter_context(tc.tile_pool(name="work", bufs=3))
        evac = ctx.enter_context(tc.tile_pool(name="evac", bufs=4))
        psg = ctx.enter_context(tc.tile_pool(name="psg", bufs=2, space="PSUM"))
        psav = ctx.enter_context(tc.tile_pool(name="psav", bufs=1, space="PSUM"))

        # ---- load inputs. xT on the sync DGE ring, W on the act ring so the
        # two streams run in parallel; per-k interleave lets the first pT
        # projection group accumulate k-chunks as they land.
        ident = res.tile([128, 128], DT, tag="I", name="I")
        nc.sync.dma_start(out=ident[:], in_=I_d[:, :])
        # preload the exp table set while input DMAs stream (first real exp
        # would otherwise pay the ~2.7us ACT_TABLE_LOAD on the critical path)
        warm = work.tile([1, 128], F32, tag="warm", name="warm", bufs=1)
        nc.scalar.activation(out=warm[:], in_=ident[0:1, :],
                             func=mybir.ActivationFunctionType.Exp, scale=1.0)
        # Inputs land in dependency order across both hwdge rings: the
        # columns the first projection groups touch come first.
        xT = [res.tile([128, S], DT, tag=f"xT{i}", name=f"xT{i}")
              for i in range(KT)]
        Wt = [res.tile([128, H], DT, tag=f"W{i}", name=f"W{i}")
              for i in range(KT)]
        for i in range(KT):
            qx = nc.sync if i % 2 == 0 else nc.scalar
            qw = nc.scalar if i % 2 == 0 else nc.sync
            qx.dma_start(out=xT[i][:, 0:1024], in_=xT_d[i * 128:(i + 1) * 128, 0:1024])
            qw.dma_start(out=Wt[i][:, 0:256], in_=W_d[i * 128:(i + 1) * 128, 0:256])
        for i in range(KT):
            qx = nc.sync if i % 2 == 0 else nc.scalar
            qw = nc.scalar if i % 2 == 0 else nc.sync
            qx.dma_start(out=xT[i][:, 1024:2048],
                         in_=xT_d[i * 128:(i + 1) * 128, 1024:2048])
            qw.dma_start(out=Wt[i][:, 256:1024],
                         in_=W_d[i * 128:(i + 1) * 128, 256:1024])

        # ---- pT = (x @ W)^T : [H, S] bf16, 8 tiles of [128, S]
        pT = [res.tile([128, S], DT, tag=f"pT{i}", name=f"pT{i}") for i in range(KT)]

        def proj_pT_group(f, sc):
            ps = psg.tile([128, 1024], F32, tag="g", name="pjg")
            for k in range(KT):
                nc.tensor.matmul(
                    out=ps[:, 0:512],
                    lhsT=Wt[k][:, f * 128:(f + 1) * 128],
                    rhs=xT[k][:, sc * 512:(sc + 1) * 512],
                    start=(k == 0),
                    stop=(k == KT - 1),
                )
            nc.vector.tensor_copy(
                out=pT[f][:, sc * 512:(sc + 1) * 512], in_=ps[:, 0:512]
            )

        # ---- p natural (values), derived from pT by PE transpose.
        # One big tile [128, ST*PPW]; tile st occupies cols st*PPW:(st+1)*PPW
        # with layout [O vA0 vB0 O vA1 vB1 ... O] (O = 64 ones cols).
        p_pad = res.tile([128, ST * PPW], DT, tag="pp", name="pp")
        for st in range(ST):
            main = p_pad[:, st * PPW:st * PPW + NP * PW].rearrange(
                "p (g u) -> p g u", u=PW
            )[:, :, 0:64]
            nc.vector.memset(main, 1.0)
            nc.vector.memset(p_pad[:, st * PPW + NP * PW:(st + 1) * PPW], 1.0)

        def transpose_pack(fp, half):
            # transpose 8 pT[fp] key-chunks (st = 8*half .. 8*half+7) into
            # their p_pad [vA|vB] slots via the DMA XBAR transpose — fully
            # off the PE/DVE engines (DMA queues are nearly idle mid-run).
            for j in range(8):
                st = 8 * half + j
                dst = p_pad[:, st * PPW + fp * PW + 64:st * PPW + fp * PW + 192]
                nc.sync.dma_start(out=dst,
                                  in_=pT[fp][:, st * 128:(st + 1) * 128],
                                  transpose=True)

        # pT[0] q-side chunks precede the loop; k-side chunks and all
        # transposes stream just-in-time inside the attention loop.
        proj_pT_group(0, 0)
        proj_pT_group(0, 1)

        # ---- attention: one continuous software-pipelined stream over all
        # (pair, kt) steps. Crossing a pair boundary, the old pair's last AV
        # accumulations and epilogue interleave into the new pair's first
        # gram/exp iterations, so the ACT engine (the bottleneck) never
        # drains. AV lags gram/exp by AV_SKEW steps.
        attnT = [res.tile([128, Q], DT, tag=f"at{i}", name=f"at{i}") for i in range(KT)]

        def do_av(eA, eB, fp, kt, av0, av1):
            st0, sp0 = (kt == 0), (kt == ST - 1)
            base = kt * PPW + fp * PW
            wA = p_pad[:, base:base + 128]            # [O | vA]
            wB = p_pad[:, base + 128:base + 256]      # [vB | O']
            nc.tensor.matmul(out=av0[:, 0:512], lhsT=wA,
                             rhs=eA[:, 0:512], start=st0, stop=sp0)
            nc.tensor.matmul(out=av1[:, 0:512], lhsT=wB,
                             rhs=eB[:, 0:512], start=st0, stop=sp0)
            nc.tensor.matmul(out=av0[:, 512:1024], lhsT=wA,
                             rhs=eA[:, 512:1024], start=st0, stop=sp0)
            nc.tensor.matmul(out=av1[:, 512:1024], lhsT=wB,
                             rhs=eB[:, 512:1024], start=st0, stop=sp0)

        def epilogue(fp, av0, av1):
            # denominators are replicated across 64 PSUM rows (the
            # ones-block columns of the AV lhsT): normalize is reciprocal +
            # multiply straight out of PSUM. reciprocal_approx_fast
            # mis-reads inputs at a nonzero base partition (reads base 0),
            # so D_B is staged down to base 0 first. (AluOpType.divide
            # fails walrus codegen - do not use.)
            rcA = work.tile([64, 1024], F32, tag="rcA", name="rcA", bufs=1)
            rcB = work.tile([64, 1024], F32, tag="rcB", name="rcB", bufs=1)
            dB = work.tile([64, 1024], F32, tag="dB", name="dB", bufs=1)
            nc.vector.reciprocal_approx_fast(out=rcA[:], in_=av0[0:64, :])
            nc.vector.tensor_tensor(out=attnT[fp][0:64, :], in0=av0[64:128, :],
                                    in1=rcA[:], op=mybir.AluOpType.mult)
            nc.vector.tensor_copy(out=dB[:], in_=av1[64:128, :])
            nc.vector.reciprocal_approx_fast(out=rcB[:], in_=dB[:])
            nc.vector.tensor_tensor(out=attnT[fp][64:128, :], in0=av1[0:64, :],
                                    in1=rcB[:], op=mybir.AluOpType.mult)

        pending = []
        av_cur = None
        for g in range(NP * ST + AV_SKEW + 1):
            fp, kt = divmod(g, ST)
            # AV for step g - AV_SKEW first: its e-inputs are long ready, so
            # these MMs stream without waits, and keeping them ahead of the
            # grams in program order avoids extra gram<->AV PE transitions.
            if len(pending) > AV_SKEW or g >= NP * ST:
                if pending:
                    args = pending.pop(0)
                    do_av(*args)
                    if args[3] == ST - 1:          # pair args[2] finished
                        epilogue(args[2], args[4], args[5])
            if g < NP * ST:
                if kt == 0:
                    # pair fp's AV accumulators. The previous pair's
                    # epilogue (the last reader of the old tiles) was
                    # emitted at step fp*ST + AV_SKEW - 1 above, before the
                    # first do_av of this pair needs the fresh tiles.
                    av_cur = (
                        psav.tile([128, 1024], F32, tag="av0", name="av0"),
                        psav.tile([128, 1024], F32, tag="av1", name="av1"),
                    )
                ks = slice(kt * 128, (kt + 1) * 128)
                tA = psg.tile([128, 1024], F32, tag="g", name="gA")
                tB = psg.tile([128, 1024], F32, tag="g", name="gB")
                nc.tensor.matmul(out=tA[:, 0:512], lhsT=pT[fp][0:64, ks],
                                 rhs=pT[fp][0:64, 0:512], start=True, stop=True)
                nc.tensor.matmul(out=tA[:, 512:1024], lhsT=pT[fp][0:64, ks],
                                 rhs=pT[fp][0:64, 512:1024], start=True, stop=True)
                eA = work.tile([128, 1024], DT, tag="eA", name="eA",
                               bufs=AV_SKEW + 2)
                nc.scalar.activation(out=eA[:], in_=tA[:],
                                     func=mybir.ActivationFunctionType.Exp,
                                     scale=SCALE)
                nc.tensor.matmul(out=tB[:, 0:512], lhsT=pT[fp][64:128, ks],
                                 rhs=pT[fp][64:128, 0:512], start=True, stop=True)
                nc.tensor.matmul(out=tB[:, 512:1024], lhsT=pT[fp][64:128, ks],
                                 rhs=pT[fp][64:128, 512:1024], start=True, stop=True)
                eB = work.tile([128, 1024], DT, tag="eB", name="eB",
                               bufs=AV_SKEW + 2)
                nc.scalar.activation(out=eB[:], in_=tB[:],
                                     func=mybir.ActivationFunctionType.Exp,
                                     scale=SCALE)
                pending.append((eA, eB, fp, kt, av_cur[0], av_cur[1]))
            # ---- fill PE slack with projection / transpose work
            if g >= NP * ST:
                continue
            if fp == 0:
                # pair 0's first AV_SKEW steps have no AV work yet - extra
                # PE slack hosts pair 1's projection early.
                if kt == 0:
                    proj_pT_group(0, 2)
                elif kt == 1:
                    transpose_pack(0, 0)
                elif kt == 2:
                    proj_pT_group(0, 3)
                elif kt == 3:
                    proj_pT_group(1, 0)
                elif kt == 5:
                    transpose_pack(0, 1)
                elif kt in (6, 8, 10):
                    proj_pT_group(1, (kt - 4) // 2)
                elif kt == 12:
                    transpose_pack(1, 0)
                elif kt == 13:
                    proj_pT_group(2, 0)
                elif kt == 14:
                    transpose_pack(1, 1)
                elif kt == 15:
                    proj_pT_group(2, 1)
            elif fp == 1:
                if kt in (2, 5):
                    proj_pT_group(2, (kt + 4) // 3)
                elif kt == 12:
                    transpose_pack(2, 0)
                elif kt == 14:
                    transpose_pack(2, 1)
            elif fp + 1 < NP:
                if kt in (1, 4, 7, 10):
                    proj_pT_group(fp + 1, (kt - 1) // 3)
                elif kt == 12:
                    transpose_pack(fp + 1, 0)
                elif kt == 14:
                    transpose_pack(fp + 1, 1)

        # ---- output projection: out[q, :] = attnc @ W
        for qt in range(Q // 128):
            for fc in range(H // 512):
                ps = psg.tile([128, 1024], F32, tag="g", name="opg")
                for k in range(KT):
                    nc.tensor.matmul(
                        out=ps[:, 0:512],
                        lhsT=attnT[k][:, qt * 128:(qt + 1) * 128],
                        rhs=Wt[k][:, fc * 512:(fc + 1) * 512],
                        start=(k == 0),
                        stop=(k == KT - 1),
                    )
                ot = evac.tile([128, 512], F32, tag="ot")
                nc.vector.tensor_copy(out=ot[:], in_=ps[:, 0:512])
                nc.sync.dma_start(
                    out=out_d[qt * 128:(qt + 1) * 128, fc * 512:(fc + 1) * 512],
                    in_=ot[:],
                )
    nc.finalize()
    return nc


def _get_nc():
    if "nc" not in _CACHE:
        _CACHE["nc"] = _build()
    return _CACHE["nc"]


def _install_ntff_hook():
    """Register the axon NTFF profiling hook if this image's antenv lacks
    ``axon_hooks`` (test/profiling path only; grading never hits this)."""
    import types

    try:
        from antenv.axon_hooks import get_axon_ntff_profile_hook  # noqa: F401
        return
    except ImportError:
        pass
    import antenv

    mod = types.ModuleType("antenv.axon_hooks")
    state = {"hook": None}
    mod.set_axon_ntff_profile_hook = lambda h: state.__setitem__("hook", h)
    mod.get_axon_ntff_profile_hook = lambda: state["hook"]
    sys.modules["antenv.axon_hooks"] = mod
    antenv.axon_hooks = mod
    try:
        from trn_agent_boot.trn_boot import _ntff_profile_via_ctypes

        hook = _ntff_profile_via_ctypes("/opt/axon/libaxon_pjrt.so")
        mod.set_axon_ntff_profile_hook(hook)
    except Exception as e:  # degrade: tracing skipped, run still works
        print(f"ntff hook install failed: {e}", file=sys.stderr)


def _run(x, W, trace=False):
    if trace:
        _install_ntff_hook()
    nc = _get_nc()
    bf = ml_dtypes.bfloat16
    Wb = np.ascontiguousarray(W.astype(bf))
    Ib = np.eye(128, dtype=bf)
    in_maps = []
    for c in range(8):
        b, half = divmod(c, 2)
        key = ("xT", b, half)
        if key not in _CACHE:
            xTb = np.ascontiguousarray(x[b].T).astype(bf)
            if half:
                xTb = np.ascontiguousarray(np.roll(xTb, -Q, axis=1))
            _CACHE[key] = xTb
        in_maps.append({"xT": _CACHE[key], "W": Wb, "I": Ib})
    try:
        r = run_bass_kernel_spmd(
            nc, in_maps, core_ids=list(range(8)), trace=trace
        )
    finally:
        for c in range(8):
            _CACHE.pop(("xT", c // 2, c % 2), None)
    y = np.empty((B, S, H), np.float32)
    for c in range(8):
        b, half = divmod(c, 2)
        y[b, half * Q:(half + 1) * Q, :] = r.results[c]["out"]
    _CACHE["last_result"] = r
    return y


def kernel(x, W):
    return _run(np.asarray(x, dtype=np.float32), np.asarray(W, dtype=np.float32),
                trace=bool(os.environ.get("BASS_KERNEL_TRACE")))


# revision 28
# speedup vs baseline: 1.0751x; 1.0575x over previous
"""Distributed Trainium2 kernel for nn_Attention_14181982012033.

Math (reference): p = x @ W; per-head ph = split(p); q = ph/sqrt(d);
logits = q @ ph^T; w = softmax(logits); attn = w @ ph; out = merge(attn) @ W.
Shapes: x [4, 2048, 1024] f32, W [1024, 1024] f32, 16 heads, d = 64.

Sharding (zero collectives): 8 cores = 4 batches x 2 query-halves. Each core
receives xT = x[b]^T (bf16, host-pre-transposed) and W (bf16), computes the
full projection pT = (xW)^T for its batch, derives the natural-layout values
from pT via PE identity-matmul transposes (instead of a second full
projection), runs attention for its 1024 query rows over all 2048 keys,
applies the output projection, and writes a [1024, 1024] f32 slab.

SPMD trick: both cores of a batch run the IDENTICAL graph. Core 2b+1's xT is
rolled by -1024 along S, so "query rows" are always pT[:, 0:1024]; softmax
over keys is permutation-invariant, so rolled keys give identical output.

The attention inner loop is ACT(exp)-bound (~2.3 us/kt: 2 exps of
[128,1024] at ~1.15 us vs ~1.8 us of PE matmul+fill work), so all PE-side
projection/transpose work is interleaved into the loop's slack.

Softmax denominator trick: p_pad tiles are laid out [O vA vB][O vA vB]...[O]
with O = 64 ones columns shared across adjacent pairs. Head A's AV lhsT is
[O|vA] (denominator lands replicated in PSUM rows 0:64, numerators in
64:128); head B's is [vB|O'] (numerators 0:64, denominator 64:128). The
epilogue is then just two DVE ops per pair (no DRAM roundtrip, no
broadcast): attnT = numer * reciprocal(denom-replicated).
"""

import os
import sys
from contextlib import ExitStack

import numpy as np

for _p in ("/opt/trn_rl_repo", "/opt/pypackages"):
    if _p not in sys.path:
        sys.path.append(_p)

import ml_dtypes

import concourse.bass as bass
import concourse.bacc as bacc
import concourse.mybir as mybir
import concourse.tile as tile
from concourse.bass_utils import run_bass_kernel_spmd

B, S, H, NH, D = 4, 2048, 1024, 16, 64
Q = 1024          # query rows per core
KT = H // 128     # 8 partition tiles along H
ST = S // 128     # 16 partition tiles along S
NP = NH // 2      # 8 head-pairs
PW = 192          # pair stride in p_pad: [ones(64) | vA(64) | vB(64)]
PPW = NP * PW + 64  # 1600 cols per p_pad tile (trailing ones block)
DT = mybir.dt.bfloat16
F32 = mybir.dt.float32
SCALE = 1.0 / float(np.sqrt(D))
AV_SKEW = 4

_CACHE = {}


def _build():
    nc = bacc.Bacc()
    xT_d = nc.declare_dram_parameter("xT", [H, S], DT, isOutput=False)
    W_d = nc.declare_dram_parameter("W", [H, H], DT, isOutput=False)
    I_d = nc.declare_dram_parameter("I", [128, 128], DT, isOutput=False)
    out_d = nc.declare_dram_parameter("out", [Q, H], F32, isOutput=True)

    with ExitStack() as ctx:
        tc = ctx.enter_context(tile.TileContext(nc))
        res = ctx.enter_context(tc.tile_pool(name="res", bufs=1))
        work = ctx.enter_context(tc.tile_pool(name="work", bufs=3))
        evac = ctx.enter_context(tc.tile_pool(name="evac", bufs=4))
        psg = ctx.enter_context(tc.tile_pool(name="psg", bufs=2, space="PSUM"))
        psav = ctx.enter_context(tc.tile_pool(name="psav", bufs=1, space="PSUM"))

        # ---- load inputs. xT on the sync DGE ring, W on the act ring so the
        # two streams run in parallel; per-k interleave lets the first pT
        # projection group accumulate k-chunks as they land.
        ident = res.tile([128, 128], DT, tag="I", name="I")
        nc.sync.dma_start(out=ident[:], in_=I_d[:, :])
        # preload the exp table set while input DMAs stream (first real exp
        # would otherwise pay the ~2.7us ACT_TABLE_LOAD on the critical path)
        warm = work.tile([1, 128], F32, tag="warm", name="warm", bufs=1)
        nc.scalar.activation(out=warm[:], in_=ident[0:1, :],
                             func=mybir.ActivationFunctionType.Exp, scale=1.0)
        # Inputs land in dependency order across both hwdge rings: the
        # columns the first projection groups touch come first.
        xT = [res.tile([128, S], DT, tag=f"xT{i}", name=f"xT{i}")
              for i in range(KT)]
        Wt = [res.tile([128, H], DT, tag=f"W{i}", name=f"W{i}")
              for i in range(KT)]
        for i in range(KT):
            qx = nc.sync if i % 2 == 0 else nc.scalar
            qw = nc.scalar if i % 2 == 0 else nc.sync
            qx.dma_start(out=xT[i][:, 0:1024], in_=xT_d[i * 128:(i + 1) * 128, 0:1024])
            qw.dma_start(out=Wt[i][:, 0:256], in_=W_d[i * 128:(i + 1) * 128, 0:256])
        for i in range(KT):
            qx = nc.sync if i % 2 == 0 else nc.scalar
            qw = nc.scalar if i % 2 == 0 else nc.sync
            qx.dma_start(out=xT[i][:, 1024:2048],
                         in_=xT_d[i * 128:(i + 1) * 128, 1024:2048])
            qw.dma_start(out=Wt[i][:, 256:1024],
                         in_=W_d[i * 128:(i + 1) * 128, 256:1024])

        # ---- pT = (x @ W)^T : [H, S] bf16, 8 tiles of [128, S]
        pT = [res.tile([128, S], DT, tag=f"pT{i}", name=f"pT{i}") for i in range(KT)]

        def proj_pT_group(f, sc):
            ps = psg.tile([128, 1024], F32, tag="g", name="pjg")
            for k in range(KT):
                nc.tensor.matmul(
                    out=ps[:, 0:512],
                    lhsT=Wt[k][:, f * 128:(f + 1) * 128],
                    rhs=xT[k][:, sc * 512:(sc + 1) * 512],
                    start=(k == 0),
                    stop=(k == KT - 1),
                )
            nc.vector.tensor_copy(
                out=pT[f][:, sc * 512:(sc + 1) * 512], in_=ps[:, 0:512]
            )

        # ---- p natural (values), derived from pT by PE transpose.
        # One big tile [128, ST*PPW]; tile st occupies cols st*PPW:(st+1)*PPW
        # with layout [O vA0 vB0 O vA1 vB1 ... O] (O = 64 ones cols).
        p_pad = res.tile([128, ST * PPW], DT, tag="pp", name="pp")
        for st in range(ST):
            main = p_pad[:, st * PPW:st * PPW + NP * PW].rearrange(
                "p (g u) -> p g u", u=PW
            )[:, :, 0:64]
            nc.vector.memset(main, 1.0)
            nc.vector.memset(p_pad[:, st * PPW + NP * PW:(st + 1) * PPW], 1.0)

        def transpose_pack(fp, half):
            # transpose 8 pT[fp] key-chunks (st = 8*half .. 8*half+7) into
            # their p_pad [vA|vB] slots via the DMA XBAR transpose — fully
            # off the PE/DVE engines (DMA queues are nearly idle mid-run).
            for j in range(8):
                st = 8 * half + j
                dst = p_pad[:, st * PPW + fp * PW + 64:st * PPW + fp * PW + 192]
                nc.sync.dma_start(out=dst,
                                  in_=pT[fp][:, st * 128:(st + 1) * 128],
                                  transpose=True)

        # pT[0] q-side chunks precede the loop; k-side chunks and all
        # transposes stream just-in-time inside the attention loop.
        proj_pT_group(0, 0)
        proj_pT_group(0, 1)

        # ---- attention: one continuous software-pipelined stream over all
        # (pair, kt) steps. Crossing a pair boundary, the old pair's last AV
        # accumulations and epilogue interleave into the new pair's first
        # gram/exp iterations, so the ACT engine (the bottleneck) never
        # drains. AV lags gram/exp by AV_SKEW steps.
        attnT = [res.tile([128, Q], DT, tag=f"at{i}", name=f"at{i}") for i in range(KT)]

        def do_av(e0, e1, fp, kt, av0, av1):
            # e0 = [expA q0:512 | expB q512:1024], e1 = [expB q0:512 |
            # expA q512:1024] (cross-wired so each exp depends on one gram
            # matmul from EACH PE row group - see gram emission below).
            st0, sp0 = (kt == 0), (kt == ST - 1)
            base = kt * PPW + fp * PW
            wA = p_pad[:, base:base + 128]            # [O | vA]
            wB = p_pad[:, base + 128:base + 256]      # [vB | O']
            nc.tensor.matmul(out=av0[:, 0:512], lhsT=wA,
                             rhs=e0[:, 0:512], start=st0, stop=sp0)
            nc.tensor.matmul(out=av1[:, 0:512], lhsT=wB,
                             rhs=e1[:, 0:512], start=st0, stop=sp0)
            nc.tensor.matmul(out=av0[:, 512:1024], lhsT=wA,
                             rhs=e1[:, 512:1024], start=st0, stop=sp0)
            nc.tensor.matmul(out=av1[:, 512:1024], lhsT=wB,
                             rhs=e0[:, 512:1024], start=st0, stop=sp0)

        def epilogue(fp, av0, av1):
            # denominators are replicated across 64 PSUM rows (the
            # ones-block columns of the AV lhsT): normalize is reciprocal +
            # multiply straight out of PSUM. reciprocal_approx_fast
            # mis-reads inputs at a nonzero base partition (reads base 0),
            # so D_B is staged down to base 0 first. (AluOpType.divide
            # fails walrus codegen - do not use.)
            rcA = work.tile([64, 1024], F32, tag="rcA", name="rcA", bufs=1)
            rcB = work.tile([64, 1024], F32, tag="rcB", name="rcB", bufs=1)
            dB = work.tile([64, 1024], F32, tag="dB", name="dB", bufs=1)
            nc.vector.reciprocal_approx_fast(out=rcA[:], in_=av0[0:64, :])
            nc.vector.tensor_tensor(out=attnT[fp][0:64, :], in0=av0[64:128, :],
                                    in1=rcA[:], op=mybir.AluOpType.mult)
            nc.vector.tensor_copy(out=dB[:], in_=av1[64:128, :])
            nc.vector.reciprocal_approx_fast(out=rcB[:], in_=dB[:])
            nc.vector.tensor_tensor(out=attnT[fp][64:128, :], in0=av1[0:64, :],
                                    in1=rcB[:], op=mybir.AluOpType.mult)

        pending = []
        av_cur = None
        for g in range(NP * ST + AV_SKEW + 1):
            fp, kt = divmod(g, ST)
            # AV for step g - AV_SKEW first: its e-inputs are long ready, so
            # these MMs stream without waits, and keeping them ahead of the
            # grams in program order avoids extra gram<->AV PE transitions.
            if len(pending) > AV_SKEW or g >= NP * ST:
                if pending:
                    args = pending.pop(0)
                    do_av(*args)
                    if args[3] == ST - 1:          # pair args[2] finished
                        epilogue(args[2], args[4], args[5])
            if g < NP * ST:
                if kt == 0:
                    # pair fp's AV accumulators. The previous pair's
                    # epilogue (the last reader of the old tiles) was
                    # emitted at step fp*ST + AV_SKEW - 1 above, before the
                    # first do_av of this pair needs the fresh tiles.
                    av_cur = (
                        psav.tile([128, 1024], F32, tag="av0", name="av0"),
                        psav.tile([128, 1024], F32, tag="av1", name="av1"),
                    )
                ks = slice(kt * 128, (kt + 1) * 128)
                # Each gram PSUM tile mixes the two heads (= the two PE row
                # groups h0/h64): its exp then depends on one matmul from
                # each group, so the scheduler's "start the exp earliest"
                # greed places cross-row-group matmuls adjacent, letting
                # the 64x128 row tiles stream concurrently.
                t0 = psg.tile([128, 1024], F32, tag="g", name="g0")
                t1 = psg.tile([128, 1024], F32, tag="g", name="g1")
                nc.tensor.matmul(out=t0[:, 0:512], lhsT=pT[fp][0:64, ks],
                                 rhs=pT[fp][0:64, 0:512], start=True, stop=True)
                nc.tensor.matmul(out=t0[:, 512:1024], lhsT=pT[fp][64:128, ks],
                                 rhs=pT[fp][64:128, 512:1024], start=True, stop=True)
                e0 = work.tile([128, 1024], DT, tag="eA", name="eA",
                               bufs=AV_SKEW + 2)
                nc.scalar.activation(out=e0[:], in_=t0[:],
                                     func=mybir.ActivationFunctionType.Exp,
                                     scale=SCALE)
                nc.tensor.matmul(out=t1[:, 0:512], lhsT=pT[fp][64:128, ks],
                                 rhs=pT[fp][64:128, 0:512], start=True, stop=True)
                nc.tensor.matmul(out=t1[:, 512:1024], lhsT=pT[fp][0:64, ks],
                                 rhs=pT[fp][0:64, 512:1024], start=True, stop=True)
                e1 = work.tile([128, 1024], DT, tag="eB", name="eB",
                               bufs=AV_SKEW + 2)
                nc.scalar.activation(out=e1[:], in_=t1[:],
                                     func=mybir.ActivationFunctionType.Exp,
                                     scale=SCALE)
                pending.append((e0, e1, fp, kt, av_cur[0], av_cur[1]))
            # ---- fill PE slack with projection / transpose work
            if g >= NP * ST:
                continue
            if fp == 0:
                # pair 0's first AV_SKEW steps have no AV work yet - extra
                # PE slack hosts pair 1's projection early.
                if kt == 0:
                    proj_pT_group(0, 2)
                elif kt == 1:
                    transpose_pack(0, 0)
                elif kt == 2:
                    proj_pT_group(0, 3)
                elif kt == 3:
                    proj_pT_group(1, 0)
                elif kt == 5:
                    transpose_pack(0, 1)
                elif kt in (6, 8, 10):
                    proj_pT_group(1, (kt - 4) // 2)
                elif kt == 12:
                    transpose_pack(1, 0)
                elif kt == 13:
                    proj_pT_group(2, 0)
                elif kt == 14:
                    transpose_pack(1, 1)
                elif kt == 15:
                    proj_pT_group(2, 1)
            elif fp == 1:
                if kt in (2, 5):
                    proj_pT_group(2, (kt + 4) // 3)
                elif kt == 12:
                    transpose_pack(2, 0)
                elif kt == 14:
                    transpose_pack(2, 1)
            elif fp + 1 < NP:
                if kt in (1, 4, 7, 10):
                    proj_pT_group(fp + 1, (kt - 1) // 3)
                elif kt == 12:
                    transpose_pack(fp + 1, 0)
                elif kt == 14:
                    transpose_pack(fp + 1, 1)

        # ---- output projection: out[q, :] = attnc @ W
        for qt in range(Q // 128):
            for fc in range(H // 512):
                ps = psg.tile([128, 1024], F32, tag="g", name="opg")
                for k in range(KT):
                    nc.tensor.matmul(
                        out=ps[:, 0:512],
                        lhsT=attnT[k][:, qt * 128:(qt + 1) * 128],
                        rhs=Wt[k][:, fc * 512:(fc + 1) * 512],
                        start=(k == 0),
                        stop=(k == KT - 1),
                    )
                ot = evac.tile([128, 512], F32, tag="ot")
                nc.vector.tensor_copy(out=ot[:], in_=ps[:, 0:512])
                nc.sync.dma_start(
                    out=out_d[qt * 128:(qt + 1) * 128, fc * 512:(fc + 1) * 512],
                    in_=ot[:],
                )
    nc.finalize()
    return nc


def _get_nc():
    if "nc" not in _CACHE:
        _CACHE["nc"] = _build()
    return _CACHE["nc"]


def _install_ntff_hook():
    """Register the axon NTFF profiling hook if this image's antenv lacks
    ``axon_hooks`` (test/profiling path only; grading never hits this)."""
    import types

    try:
        from antenv.axon_hooks import get_axon_ntff_profile_hook  # noqa: F401
        return
    except ImportError:
        pass
    import antenv

    mod = types.ModuleType("antenv.axon_hooks")
    state = {"hook": None}
    mod.set_axon_ntff_profile_hook = lambda h: state.__setitem__("hook", h)
    mod.get_axon_ntff_profile_hook = lambda: state["hook"]
    sys.modules["antenv.axon_hooks"] = mod
    antenv.axon_hooks = mod
    try:
        from trn_agent_boot.trn_boot import _ntff_profile_via_ctypes

        hook = _ntff_profile_via_ctypes("/opt/axon/libaxon_pjrt.so")
        mod.set_axon_ntff_profile_hook(hook)
    except Exception as e:  # degrade: tracing skipped, run still works
        print(f"ntff hook install failed: {e}", file=sys.stderr)


def _run(x, W, trace=False):
    if trace:
        _install_ntff_hook()
    nc = _get_nc()
    bf = ml_dtypes.bfloat16
    Wb = np.ascontiguousarray(W.astype(bf))
    Ib = np.eye(128, dtype=bf)
    in_maps = []
    for c in range(8):
        b, half = divmod(c, 2)
        key = ("xT", b, half)
        if key not in _CACHE:
            xTb = np.ascontiguousarray(x[b].T).astype(bf)
            if half:
                xTb = np.ascontiguousarray(np.roll(xTb, -Q, axis=1))
            _CACHE[key] = xTb
        in_maps.append({"xT": _CACHE[key], "W": Wb, "I": Ib})
    try:
        r = run_bass_kernel_spmd(
            nc, in_maps, core_ids=list(range(8)), trace=trace
        )
    finally:
        for c in range(8):
            _CACHE.pop(("xT", c // 2, c % 2), None)
    y = np.empty((B, S, H), np.float32)
    for c in range(8):
        b, half = divmod(c, 2)
        y[b, half * Q:(half + 1) * Q, :] = r.results[c]["out"]
    _CACHE["last_result"] = r
    return y


def kernel(x, W):
    return _run(np.asarray(x, dtype=np.float32), np.asarray(W, dtype=np.float32),
                trace=bool(os.environ.get("BASS_KERNEL_TRACE")))
